# revision 46
# baseline (speedup 1.0000x reference)
"""Trainium2 Bass kernel for windowed Conv1d(k=3) + sigmoid gating.

Reference computation (B=16, T=960, D=1024, W=10):
  windows of size 10 are conv'd independently with per-window zero pad 1:
    cnn[t, d] = sum_{k,c} conv_w[d, c, k] * xpad[t + k, c] + conv_b[d]
    out = cnn * sigmoid(cnn @ gate_w.T + gate_b)

Strategy: pure data parallelism over the 8 NeuronCores (2 batches per
core, 192 windows = 1920 rows each). The conv runs as Winograd F(5,3)
with points {0, +-1, +-2, 1/2, inf}: each 12-long padded window splits
into two 7-point tiles (outputs t=0..4 and t=5..9) sharing the same 7
transformed weights, so each of the 7 product streams is one
128-contraction matmul chain over all 2*192=384 tile-columns. That is
14 multiplies per 10 outputs vs 30 direct (2.14x FLOP reduction) and
keeps every matmul at N=384 where the per-matmul LDWEIGHTS (~98 ns)
stays hidden under the 160 ns column stream.

The input/weight transforms run on the host in f32 (cast bf16); the
A^T combine runs on ScalarE (scaled copies) + VectorE (adds) with f32
intermediates, overlapped under the next d-chunk's matmul stream. The
gate is a dense 1024x1024 bf16 matmul over the combined cnn tiles,
identical to the direct version. PSUM: 5 rotating m-banks + 3 gate
banks. DMA order is arranged so the first real matmul starts ~12us in
(dck0 weights j-sliced, xt per-j), with a sized warm-up matmul burst
covering the DMA wait to keep the PE HAM clock at 8/8.
"""

import numpy as np
import ml_dtypes

import concourse.bacc as bacc
import concourse.bass as bass
import concourse.tile as tile
from concourse import mybir
from concourse.bass_utils import run_bass_kernel_spmd

BF16 = ml_dtypes.bfloat16

B, T, D, W = 16, 960, 1024, 10
NCORES = 8
BC = B // NCORES            # batches per core (2)
NWIN = BC * T // W          # windows per core (192)
RC = NWIN * W               # output rows per core (1920)
PW = W + 2                  # padded window length (12)
NJ = 7                      # winograd products per tile F(5,3)
NT = 2                      # tiles per window (t=0..4, t=5..9)
M5 = 5                      # outputs per tile
NCOL = NT * NWIN            # matmul stream columns (384)
NCH = D // 128              # 128-partition chunks of D (8)
GCH = 5                     # gate column chunks (aligned to y-tiles)
GN = RC // GCH              # gate chunk width (384)
AF = mybir.ActivationFunctionType
PTS = (0.0, 1.0, -1.0, 2.0, -2.0, 0.5)


def _winograd_mats():
    """Toom-Cook F(5,3), points PTS + infinity, pure-Vandermonde A^T:
      y_k = m0*[k==0] + (m1 +- m2) + 2^k(m3 +- m4) + m5/2^k + m6*[k==4]
    Solves for B^T from the bilinear identity (exact)."""
    m, r = M5, 3
    n = m + r - 1
    pts = PTS
    f = [np.prod([pi - pj for pj in pts if pj != pi]) for pi in pts]
    G = np.zeros((n, r))
    for i, p in enumerate(pts):
        G[i] = [p ** j / f[i] for j in range(r)]
    G[n - 1] = [0, 0, 1]
    AT = np.zeros((m, n))
    for i, p in enumerate(pts):
        for k in range(m):
            AT[k, i] = p ** k
    AT[m - 1, n - 1] = 1.0
    M = np.zeros((m * r, n))
    for i in range(n):
        for k in range(m):
            for j in range(r):
                M[k * r + j, i] = AT[k, i] * G[i, j]
    BT = np.zeros((n, n))
    for a in range(n):
        t = np.zeros(m * r)
        for k in range(m):
            for j in range(r):
                t[k * r + j] = 1.0 if a == k + j else 0.0
        sol, *_ = np.linalg.lstsq(M, t, rcond=None)
        assert np.abs(M @ sol - t).max() < 1e-9
        BT[:, a] = sol
    return G, BT, AT


_G, _BT, _AT = _winograd_mats()


def _build():
    nc = bacc.Bacc("TRN2", target_bir_lowering=False, debug=False)

    # xt[j]: [cc, ck*NCOL + (tile*NWIN + win)] transformed input, bf16
    xt = nc.dram_tensor("xt", [NJ, 128, NCH * NCOL], mybir.dt.bfloat16,
                        kind="ExternalInput")
    # cwr[dck]: [cc, (j*NCH + ck)*128 + dd] winograd conv lhsT blocks
    cwr = nc.dram_tensor("cwr", [NCH, 128, NJ * NCH * 128], mybir.dt.bfloat16,
                         kind="ExternalInput")
    # gwr[eck]: [dd, dck*128 + ee] gate lhsT blocks
    gwr = nc.dram_tensor("gwr", [NCH, 128, NCH * 128], mybir.dt.bfloat16,
                         kind="ExternalInput")
    cb = nc.dram_tensor("cb", [128, NCH], mybir.dt.float32, kind="ExternalInput")
    gb = nc.dram_tensor("gb", [128, NCH], mybir.dt.float32, kind="ExternalInput")
    outT = nc.dram_tensor("outT", [D, RC], mybir.dt.float32, kind="ExternalOutput")

    with tile.TileContext(nc) as tc:
        with (
            tc.tile_pool(name="consts", bufs=1) as consts,
            tc.tile_pool(name="cwp", bufs=3) as cwp,
            tc.tile_pool(name="work", bufs=1) as work,
            tc.tile_pool(name="gwork", bufs=2) as gwork,
            tc.tile_pool(name="psum", bufs=1, space="PSUM") as psum,
        ):
            # ---- DMA schedule (single Sync HWDGE queue, issue order =
            # transfer order). dck0's weights go j-sliced, interleaved
            # with the per-j xt loads, so the first (dck0, j0) matmul
            # chain starts after ~1MB instead of ~7.3MB.
            xt_sb = []
            for j in range(NJ):
                t = consts.tile([128, NCH * NCOL], mybir.dt.bfloat16,
                                tag=f"xt{j}")
                xt_sb.append(t)
            # single Sync HWDGE queue: the per-core HBM read path (~320
            # GB/s) is shared by both HWDGE queues, and splitting the
            # stream across queues interleaves transfers and delays the
            # first-needed bytes (measured +12us). The first TRIO of
            # d-chunks is processed j-interleaved (see below), their
            # weight slices riding between the xt loads; weights live in
            # per-j tiles (freed as soon as that j-chain retires) so the
            # rotating pool stays small enough for a 4-wide interleave.
            TRIO = 4  # d-chunks interleaved during the xt supply phase
            cwj = [[None] * NJ for _ in range(NCH)]

            def load_cw(d, j):
                t = cwp.tile([128, NCH * 128], mybir.dt.bfloat16, tag="cwj",
                             name=f"cw{d}_{j}", bufs=14)
                nc.sync.dma_start(
                    t[:], cwr[d][:, j * NCH * 128:(j + 1) * NCH * 128])
                cwj[d][j] = t

            H = NCH // 2 * NCOL
            for j in range(NJ):
                # xt halved so the first 4 ck-blocks + cw0 slice unlock
                # the d0 chain at the earliest possible byte
                nc.sync.dma_start(xt_sb[j][:, :H], xt[j][:, :H])
                load_cw(0, j)
                nc.sync.dma_start(xt_sb[j][:, H:], xt[j][:, H:])
                for d in range(1, TRIO):
                    load_cw(d, j)
                if j == 0:
                    cb_sb = consts.tile([128, NCH], mybir.dt.float32, tag="cb")
                    nc.sync.dma_start(cb_sb[:], cb[:])
                    gb_sb = consts.tile([128, NCH], mybir.dt.float32, tag="gb")
                    nc.sync.dma_start(gb_sb[:], gb[:])
            for dck in range(TRIO, NCH):
                for j in range(NJ):
                    load_cw(dck, j)
            gwr_sb = []
            for eck in range(NCH):
                t = consts.tile([128, NCH * 128], mybir.dt.bfloat16,
                                tag=f"gw{eck}")
                nc.sync.dma_start(t[:], gwr[eck])
                gwr_sb.append(t)

            # ---- warm-up: ~12 cold matmuls (~400ns each at K=4/8) bridge
            # the input-DMA wait and flip the PE HAM clock gate to 8/8
            # just as the real stream starts.
            scr = consts.tile([128, 512], mybir.dt.bfloat16, tag="scr")
            nc.gpsimd.memset(scr[:], 0.0)
            for _ in range(13):
                wps = psum.tile([128, 512], mybir.dt.float32, tag="g", bufs=3)
                nc.tensor.matmul(wps[:, :480], scr[:, :128], scr[:, :480],
                                 start=True, stop=True)

            # ---- conv: per d-chunk, 7 winograd product streams of
            # N=384, then the A^T combine on ScalarE/VectorE.
            ct_sb = []
            for dck in range(NCH):
                t = consts.tile([128, M5 * NCOL], mybir.dt.bfloat16,
                                tag=f"ct{dck}")
                ct_sb.append(t)

            MUL = mybir.AluOpType.mult
            ADD = mybir.AluOpType.add

            def wtile(name, bufs):
                return work.tile([128, NCOL], mybir.dt.float32, tag=name,
                                 name=name, bufs=bufs)

            # Eager A^T combine, one stage per product stream j, so each
            # m PSUM bank frees within 1-2 ops of its chain finishing.
            # This lets the supply-bound phase interleave TRIO d-chunks
            # (j-major) with only ~3 live m banks:
            #   y0 = m0+s1+s2+c5+cb       y1 = d1+2d2+c5/2
            #   y2 = s1+4s2+c5/4          y3 = d1+8d2+c5/8
            #   y4 = s1+16s2+c5/16+m6     (cb folded into s1/d1)
            st = [dict() for _ in range(NCH)]

            def stage(d, j, m):
                s = st[d]
                cbs = cb_sb[:, d:d + 1]
                ct = ct_sb[d]
                ctv = [ct[:, k * NCOL:(k + 1) * NCOL] for k in range(M5)]
                if j == 0:
                    s['a0'] = wtile("a0", TRIO)
                    nc.scalar.activation(s['a0'][:], m, AF.Copy)
                elif j == 1:
                    s['a1'] = wtile("a1", TRIO)
                    nc.scalar.activation(s['a1'][:], m, AF.Copy)
                elif j == 2:
                    c2p = wtile("c2p", 1)
                    nc.scalar.activation(c2p[:], m, AF.Identity, bias=cbs)
                    c2m = wtile("c2m", 1)
                    nc.scalar.activation(c2m[:], m, AF.Identity, bias=cbs,
                                         scale=-1.0)
                    s['s1'] = wtile("s1", TRIO)
                    nc.vector.tensor_add(s['s1'][:], s['a1'][:], c2p[:])
                    s['d1'] = wtile("d1", TRIO)
                    nc.vector.tensor_add(s['d1'][:], s['a1'][:], c2m[:])
                elif j == 3:
                    s['a3'] = wtile("a3", TRIO)
                    nc.scalar.activation(s['a3'][:], m, AF.Copy)
                elif j == 4:
                    c4 = wtile("c4", 1)
                    nc.scalar.activation(c4[:], m, AF.Copy)
                    s['s2'] = wtile("s2", TRIO)
                    nc.vector.tensor_add(s['s2'][:], s['a3'][:], c4[:])
                    s['d2'] = wtile("d2", TRIO)
                    nc.vector.tensor_sub(s['d2'][:], s['a3'][:], c4[:])
                    # t0's inputs are ready since j2; computing it here
                    # shortens y0's critical path after the j5 chain
                    s['t0'] = wtile("t0", 2)
                    nc.vector.tensor_add(s['t0'][:], s['a0'][:], s['s1'][:])
                elif j == 5:
                    c5 = s['c5'] = wtile("c5", TRIO)
                    nc.scalar.activation(c5[:], m, AF.Copy)
                    s1, d1 = s['s1'], s['d1']
                    s2, d2 = s['s2'], s['d2']
                    # y-chains split across Vector and GpSimd (both
                    # "either-vector" engines; all operands SBUF) so the
                    # TRIO's combined backlog doesn't hold up the m-pool
                    t1 = wtile("t1", 2)
                    nc.vector.tensor_add(t1[:], s2[:], c5[:])
                    nc.vector.tensor_add(ctv[0], s['t0'][:], t1[:])
                    qa = wtile("qa", 1)
                    nc.vector.scalar_tensor_tensor(qa[:], d2[:], 2.0, d1[:],
                                                   MUL, ADD)
                    nc.vector.scalar_tensor_tensor(ctv[1], c5[:], 0.5, qa[:],
                                                   MUL, ADD)
                    qb = wtile("qb", 1)
                    nc.vector.scalar_tensor_tensor(qb[:], s2[:], 4.0, s1[:],
                                                   MUL, ADD)
                    nc.vector.scalar_tensor_tensor(ctv[2], c5[:], 0.25, qb[:],
                                                   MUL, ADD)
                    qc = wtile("qa", 1)
                    nc.vector.scalar_tensor_tensor(qc[:], d2[:], 8.0, d1[:],
                                                   MUL, ADD)
                    nc.vector.scalar_tensor_tensor(ctv[3], c5[:], 0.125,
                                                   qc[:], MUL, ADD)
                    # q4/q5 need only s2/s1/c5 — run them at j5 so j6's
                    # y4 is the single op freeing the m6 bank
                    qd = wtile("qb", 1)
                    nc.vector.scalar_tensor_tensor(qd[:], s2[:], 16.0,
                                                   s1[:], MUL, ADD)
                    s['qe'] = wtile("qe", TRIO)
                    nc.vector.scalar_tensor_tensor(s['qe'][:], c5[:], 0.0625,
                                                   qd[:], MUL, ADD)
                else:
                    # m6 stashed via ScalarE so every m bank is freed by
                    # Scalar — the PE never waits on the Vector backlog
                    a6 = wtile("a6", 2)
                    nc.scalar.activation(a6[:], m, AF.Copy)
                    nc.vector.tensor_add(ctv[4], a6[:], s['qe'][:])

            groups = [tuple(range(TRIO))] + [(d,) for d in range(TRIO, NCH)]
            for group in groups:
                for j in range(NJ):
                    for d in group:
                        mj = psum.tile([128, 512], mybir.dt.float32, tag="m",
                                       bufs=5, name=f"m{j}")
                        for ck in range(NCH):
                            nc.tensor.matmul(
                                mj[:, :NCOL],
                                cwj[d][j][:, ck * 128:(ck + 1) * 128],
                                xt_sb[j][:, ck * NCOL:(ck + 1) * NCOL],
                                start=(ck == 0),
                                stop=(ck == NCH - 1),
                            )
                        stage(d, j, mj[:, :NCOL])

            # ---- gate: gateT[e, r] = sigmoid(sum_d gw[d,e] cnnT[d,r] + gb)
            for eck in range(NCH):
                for g in range(GCH):
                    gp = psum.tile([128, 512], mybir.dt.float32, tag="g",
                                   bufs=3, name="gp")
                    for dck in range(NCH):
                        nc.tensor.matmul(
                            gp[:, :GN],
                            gwr_sb[eck][:, dck * 128:(dck + 1) * 128],
                            ct_sb[dck][:, g * GN:(g + 1) * GN],
                            start=(dck == 0),
                            stop=(dck == NCH - 1),
                        )
                    gt = gwork.tile([128, GN], mybir.dt.bfloat16, tag="gate")
                    ot = gwork.tile([128, GN], mybir.dt.float32, tag="out",
                                    bufs=4)
                    last = (eck == NCH - 1 and g == GCH - 1)
                    chunks = ((0, 256), (256, GN)) if last else ((0, GN),)
                    for lo, hi in chunks:
                        nc.scalar.activation(gt[:, lo:hi], gp[:, lo:hi],
                                             AF.Sigmoid,
                                             bias=gb_sb[:, eck:eck + 1])
                        nc.vector.tensor_mul(ot[:, lo:hi],
                                             ct_sb[eck][:, g * GN + lo:
                                                        g * GN + hi],
                                             gt[:, lo:hi])
                        nc.sync.dma_start(
                            outT[eck * 128:(eck + 1) * 128,
                                 g * GN + lo:g * GN + hi], ot[:, lo:hi])
    nc.compile()
    return nc


def _prep_core_input(x_shard, cw_host, gw_host, cb_host, gb_host):
    # x_shard: [BC, T, D] -> winograd-transformed [NJ, 128, NCH*NCOL]
    xs = x_shard.reshape(NWIN, W, D)
    xp = np.zeros((NWIN, PW, D), np.float32)
    xp[:, 1:1 + W, :] = xs
    tiles = np.stack([xp[:, 0:NJ, :], xp[:, M5:M5 + NJ, :]], axis=1)
    # xt[j, c, tile*NWIN + win]
    xt = np.einsum('jn,wtnc->jctw', _BT.astype(np.float32), tiles,
                   optimize=True).reshape(NJ, NCH, 128, NCOL)
    xt_host = np.ascontiguousarray(xt.transpose(0, 2, 1, 3)).reshape(
        NJ, 128, NCH * NCOL).astype(BF16)
    return {"xt": xt_host, "cwr": cw_host, "gwr": gw_host,
            "cb": cb_host, "gb": gb_host}


def _prep_in_maps(x, conv_w, conv_b, gate_w, gate_b):
    # winograd weight transform + lhsT blocks:
    # cwr[dck][cc, (j*NCH+ck)*128 + dd] = wt[j, dck*128+dd, ck*128+cc]
    wt = np.einsum('jn,dcn->jdc', _G.astype(np.float32), conv_w,
                   optimize=True)
    wt = wt.reshape(NJ, NCH, 128, NCH, 128)  # [j, dck, dd, ck, cc]
    cw_host = np.ascontiguousarray(wt.transpose(1, 4, 0, 3, 2)).reshape(
        NCH, 128, NJ * NCH * 128).astype(BF16)
    # gate lhsT blocks: gwr[eck][dd, dck*128 + ee] = gate_w[eck*128+ee, dck*128+dd]
    gwt = gate_w.T.reshape(NCH, 128, NCH, 128)  # [dck, dd, eck, ee]
    gw_host = np.ascontiguousarray(gwt.transpose(2, 1, 0, 3)).reshape(
        NCH, 128, NCH * 128).astype(BF16)
    cb_host = np.ascontiguousarray(conv_b.reshape(NCH, 128).T).astype(np.float32)
    gb_host = np.ascontiguousarray(gate_b.reshape(NCH, 128).T).astype(np.float32)
    return [
        _prep_core_input(x[BC * i:BC * (i + 1)], cw_host, gw_host, cb_host,
                         gb_host)
        for i in range(NCORES)
    ]


def _unshard_core(o):
    # o: [D, RC] with columns (k, tile, win=b*96+n) -> [BC, T, D]
    return (o.reshape(D, M5, NT, BC, NWIN // BC)
             .transpose(3, 4, 2, 1, 0)
             .reshape(BC, T, D))


_NC_CACHE = None


def kernel(x, conv_w, conv_b, gate_w, gate_b):
    global _NC_CACHE
    x = np.asarray(x, np.float32)
    conv_w = np.asarray(conv_w, np.float32)
    conv_b = np.asarray(conv_b, np.float32)
    gate_w = np.asarray(gate_w, np.float32)
    gate_b = np.asarray(gate_b, np.float32)

    in_maps = _prep_in_maps(x, conv_w, conv_b, gate_w, gate_b)
    if _NC_CACHE is None:
        _NC_CACHE = _build()
    res = run_bass_kernel_spmd(_NC_CACHE, in_maps, core_ids=list(range(NCORES))).results

    out = np.empty((B, T, D), np.float32)
    for i in range(NCORES):
        out[BC * i:BC * (i + 1)] = _unshard_core(np.asarray(res[i]["outT"]))
    return out
